# revision 1
# baseline (speedup 1.0000x reference)
"""GATv2-with-edge-features GNN message passing on 8 TRN2 NeuronCores.

Strategy (dst-sorted edge tiling, node-sharded across cores):
  - Host sorts edges by dst and packs them into tiles of <=128 contiguous
    dst nodes and <=768 edges.  Each of the 8 cores gets 20 tiles
    (node-disjoint), so softmax segments are tile-local and NO collectives
    are needed.
  - Device prologue (per core, replicated where needed):
      hT = relu(W_ne^T x^T)           (c-major)
      xl table[20000, 1152] = [h @ W_l | h @ (W_l att)] -> HBM (bf16)
      xr local table[2560, 1152]      (only this core's dst nodes)
  - Per tile: dma_gather xl[src], xr[dst]; e^T = relu(W_ee^T ea^T);
    s = e W_e + xl[src] + xr[dst]  (PSUM, identity-matmul adds);
    leaky_relu via 0.2*lin + 0.8*relu:  lin = att.s comes from gathered
    per-node scalars + small matmul; relu part via ACT relu + DVE
    tensor_tensor_reduce with att.
    Segment softmax via per-chunk one-hot masks (iota == dst_local),
    aggregation + denominators as dense matmuls, then the decoder MLP
    (heads mixed locally), sigmoid, and staged output.
  - Host unshards staging -> [20000, 6] float32.
"""

import os
import numpy as np
import ml_dtypes

bf16 = ml_dtypes.bfloat16

# Problem constants (hardcoded per harness contract)
N_NODES = 20000
N_EDGES = 100000
HID = 256
HEADS = 4
OUTC = 256
HC = HEADS * OUTC  # 1024
N_CORES = 8
TILES_PER_CORE = 21
NTILES = N_CORES * TILES_PER_CORE
EDGE_CAP = 640
CHUNKS = EDGE_CAP // 128  # 6
IDX_COLS = EDGE_CAP // 16  # 48
MAX_LOC = TILES_PER_CORE * 128  # 2688
NSHARD = 2560  # xl-table rows computed per core (8*2560 = 20480 padded)
TBL_W = 1152  # 1024 channels + 4 att-dot cols + pad to 256B granule (bf16)
TBL_USED = HC + HEADS  # 1028

_CACHE = {}


# --------------------------------------------------------------------------
# host-side preprocessing
# --------------------------------------------------------------------------

_xl_row_v = None  # set below (vectorized)


def _pack_idx(idx_vals):
    """Pack per-edge indices (len EDGE_CAP) into the [128, IDX_COLS] int16
    wrapped layout dma_gather expects: element i at [i % 16, i // 16]."""
    a = np.asarray(idx_vals, dtype=np.int16).reshape(IDX_COLS, 16)
    return np.tile(a.T, (8, 1))  # replicated per GPSIMD Q7 core


N_SLABS = 4
SLAB = NSHARD // N_SLABS  # 640 rows per slab per core


def _xl_row(n):
    """Global node id -> row in the slab-interleaved AllGather'd table."""
    c, rem = n // NSHARD, n % NSHARD
    s, r = rem // SLAB, rem % SLAB
    return s * (N_CORES * SLAB) + c * SLAB + r


def _sel4():
    s = np.zeros((HEADS, HEADS * 128), np.float32)
    for h in range(HEADS):
        s[h, h * 128:(h + 1) * 128] = 1.0
    return s


_xl_row_v = np.vectorize(lambda n: _xl_row(int(n)), otypes=[np.int64])


def _host_prep(inputs):
    x = np.asarray(inputs["x"], np.float32)
    ea = np.asarray(inputs["edge_attr"], np.float32)
    ei = np.asarray(inputs["edge_index"])
    src = ei[0].astype(np.int64)
    dst = ei[1].astype(np.int64)

    order = np.argsort(dst, kind="stable")
    s_s = src[order]
    d_s = dst[order]
    ea_s = ea[order]

    deg = np.bincount(dst, minlength=N_NODES)
    cum = np.concatenate([[0], np.cumsum(deg)])
    assert deg.max() <= EDGE_CAP, f"node degree {deg.max()} exceeds tile cap"

    # greedy tiles: contiguous nodes, <=128 nodes, <=EDGE_CAP edges
    tiles = []  # (n0, n1, e0, e1)
    n0 = 0
    while n0 < N_NODES:
        n1 = min(n0 + 128, N_NODES)
        while cum[n1] - cum[n0] > EDGE_CAP:
            n1 -= 1
        tiles.append((n0, n1, int(cum[n0]), int(cum[n1])))
        n0 = n1
    assert len(tiles) <= NTILES, f"{len(tiles)} tiles > capacity {NTILES}"
    while len(tiles) < NTILES:
        tiles.append((N_NODES, N_NODES, N_EDGES, N_EDGES))  # empty pads

    # weights (host layouts)
    W_ne = np.asarray(inputs["W_ne"], np.float32)
    W_ee = np.asarray(inputs["W_ee"], np.float32)
    W_l = np.asarray(inputs["W_l"], np.float32)
    W_r = np.asarray(inputs["W_r"], np.float32)
    W_e = np.asarray(inputs["W_e"], np.float32)
    att = np.asarray(inputs["att"], np.float32)
    W_d1 = np.asarray(inputs["W_d1"], np.float32)
    W_d2 = np.asarray(inputs["W_d2"], np.float32)

    b_ne = np.asarray(inputs["b_ne"], np.float32)
    b_ee = np.asarray(inputs["b_ee"], np.float32)
    b_l = np.asarray(inputs["b_l"], np.float32)
    b_r = np.asarray(inputs["b_r"], np.float32)
    b_e = np.asarray(inputs["b_e"], np.float32)
    conv_bias = np.asarray(inputs["conv_bias"], np.float32)
    b_d1 = np.asarray(inputs["b_d1"], np.float32)
    b_d2 = np.asarray(inputs["b_d2"], np.float32)

    b_tot = b_l + b_r + b_e  # enters s; split half into xl table, half xr
    b_d1p = conv_bias @ W_d1 + b_d1  # fold conv_bias into decoder bias

    # att block-diagonal contraction vectors: w_al[cin, h] = sum_c W_l[cin, h*OUTC+c] att[h, c]
    att_bd = np.zeros((HC, HEADS), np.float32)
    for h in range(HEADS):
        att_bd[h * OUTC:(h + 1) * OUTC, h] = att[h]
    # 0.2 = leaky slope folded in: lin columns come out pre-scaled
    w_al = 0.2 * (W_l @ att_bd)  # [256, 4]
    w_ar = 0.2 * (W_r @ att_bd)
    w_ae = 0.2 * (W_e @ att_bd)
    # the att-dot of the folded bias (b_tot enters tables as b_tot/2 each)
    ab_half = 0.2 * (b_tot @ att_bd) * 0.5  # [4] 0.2 * att . (b_tot/2)

    # |att|-fold + per-head sign grouping: within each head the channels are
    # permuted positive-att-first and scaled by 0.8*max(|att|, eps) so that
    # 0.8*sum_c att_c relu(s_c) = sum_pos relu(s'_c) - sum_neg relu(s'_c)
    # with s' the scaled/permuted s.  The decoder (W_d1 rows) unscales.
    perm_parts = []
    kpos = []
    for h in range(HEADS):
        pos = np.where(att[h] >= 0)[0]
        neg = np.where(att[h] < 0)[0]
        kpos.append(int(len(pos)))
        perm_parts.append(h * OUTC + np.concatenate([pos, neg]))
    P = np.concatenate(perm_parts)  # [1024]
    vscale = 0.8 * np.maximum(np.abs(att.reshape(-1)[P]), 1e-3)  # [1024]
    W_l = W_l[:, P] * vscale
    W_r = W_r[:, P] * vscale
    W_e = W_e[:, P] * vscale
    b_tot = b_tot[P] * vscale
    W_d1 = W_d1[P, :] / vscale[:, None]
    sgn = np.where(att.reshape(-1)[P] >= 0, 1.0, -1.0).astype(np.float32)

    def chunk2(w):  # [256, K] -> [128, 2, K]
        return np.ascontiguousarray(w.reshape(2, 128, -1).transpose(1, 0, 2))

    per_core_common = {

        "w_ne": W_ne.astype(bf16),  # [7, 256]
        "w_ee": W_ee.astype(bf16),
        "w_l": chunk2(W_l).astype(bf16),  # [128, 2, 1024]
        "w_r": chunk2(W_r).astype(bf16),
        "w_e": chunk2(W_e).astype(bf16),
        "w_al": chunk2(w_al).astype(bf16),  # [128, 2, 4]
        "w_ar": chunk2(w_ar).astype(bf16),
        "w_ae": chunk2(w_ae).astype(bf16),
        "w_d1": np.ascontiguousarray(
            W_d1.reshape(8, 128, 256).transpose(1, 0, 2)).astype(bf16),
        "w_d2": chunk2(W_d2).astype(bf16),  # [128, 2, 6]
        "b_ne": chunk2(b_ne.reshape(HID, 1)),  # [128, 2, 1] f32
        "b_ee": chunk2(b_ee.reshape(HID, 1)),
        "b_d1p": chunk2(b_d1p.reshape(HID, 1)),
        "i128": np.eye(128, dtype=bf16),
        "iota": np.broadcast_to(
            np.arange(128, dtype=np.float32), (128, 128)).copy(),
        "btot2": (b_tot * 0.5).reshape(1, HC).astype(np.float32),
        "abh": ab_half.reshape(1, HEADS).astype(np.float32),
        "b_d2b": np.broadcast_to(b_d2, (128, 6)).astype(np.float32).copy(),
        "sgn_b": np.broadcast_to(sgn, (128, HC)).astype(bf16).copy(),
        "sel4": _sel4(),
    }
    flags = (bool(np.any(b_tot != 0.0)), bool(np.any(b_d2 != 0.0)),
             tuple(kpos))

    in_maps = []
    meta = []  # per core: list of (n0, n1) per tile
    for c in range(N_CORES):
        ctiles = tiles[c * TILES_PER_CORE:(c + 1) * TILES_PER_CORE]
        core_n0 = ctiles[0][0]
        eaT = np.zeros((TILES_PER_CORE, 7, EDGE_CAP), bf16)
        idxl = np.zeros((TILES_PER_CORE, 128, IDX_COLS), np.int16)
        idxr = np.zeros((TILES_PER_CORE, 128, IDX_COLS), np.int16)
        dstloc = np.full((TILES_PER_CORE, 128, CHUNKS), -1.0, np.float32)
        xloc = np.zeros((MAX_LOC, 7), np.float32)
        for t, (n0, n1, e0, e1) in enumerate(ctiles):
            ne = e1 - e0
            nv = n1 - n0
            if nv > 0:
                xloc[t * 128:t * 128 + nv] = x[n0:n1]
            if ne == 0:
                continue
            eaT[t, :, :ne] = ea_s[e0:e1].T.astype(bf16)
            il = np.zeros(EDGE_CAP, np.int64)
            ir = np.zeros(EDGE_CAP, np.int64)
            il[:ne] = _xl_row_v(s_s[e0:e1])
            ir[:ne] = t * 128 + (d_s[e0:e1] - n0)
            idxl[t] = _pack_idx(il)
            idxr[t] = _pack_idx(ir)
            dl = np.full(EDGE_CAP, -1.0, np.float32)
            dl[:ne] = (d_s[e0:e1] - n0).astype(np.float32)
            dstloc[t] = dl.reshape(CHUNKS, 128).T
        m = dict(per_core_common)
        sh0 = c * NSHARD
        xsh = np.zeros((NSHARD, 7), np.float32)
        hi = min(N_NODES, sh0 + NSHARD)
        if hi > sh0:
            xsh[:hi - sh0] = x[sh0:hi]
        m["xshT"] = np.ascontiguousarray(xsh.T).astype(bf16)  # [7, 2560]
        m["xlocT"] = np.ascontiguousarray(xloc.T).astype(bf16)  # [7, 2560]
        m["eaT"] = eaT
        m["idxl"] = idxl
        m["idxr"] = idxr
        m["dstloc"] = dstloc
        in_maps.append(m)
        meta.append([(n0, n1) for (n0, n1, _, _) in ctiles])
    return in_maps, meta, flags


# --------------------------------------------------------------------------
# bass graph
# --------------------------------------------------------------------------

def _build(flags, phase=99):
    has_btot_, has_bd2_, kpos_ = flags[0], flags[1], flags[2]
    import concourse.bass as bass
    import concourse.bacc as bacc
    import concourse.mybir as mybir
    import concourse.tile as tile
    from concourse.dve_ops import TENSOR_TENSOR_REDUCE as CTTR

    dt = mybir.dt
    F32, BF16, I16 = dt.float32, dt.bfloat16, dt.int16
    AF = mybir.ActivationFunctionType
    ALU = mybir.AluOpType
    has_btot, has_bd2, kpos = flags

    nc = bacc.Bacc("TRN2", target_bir_lowering=False, debug=False,
                   enable_asserts=False, num_devices=N_CORES)

    # ---- dram parameters
    def din(name, shape, dtype):
        return nc.dram_tensor(name, shape, dtype, kind="ExternalInput")

    xshT_d = din("xshT", [7, NSHARD], BF16)
    xlocT_d = din("xlocT", [7, MAX_LOC], BF16)
    w_ne_d = din("w_ne", [7, HID], BF16)
    w_ee_d = din("w_ee", [7, HID], BF16)
    w_l_d = din("w_l", [128, 2, HC], BF16)
    w_r_d = din("w_r", [128, 2, HC], BF16)
    w_e_d = din("w_e", [128, 2, HC], BF16)
    w_al_d = din("w_al", [128, 2, HEADS], BF16)
    w_ar_d = din("w_ar", [128, 2, HEADS], BF16)
    w_ae_d = din("w_ae", [128, 2, HEADS], BF16)
    w_d1_d = din("w_d1", [128, 8, HID], BF16)
    w_d2_d = din("w_d2", [128, 2, 6], BF16)
    b_ne_d = din("b_ne", [128, 2, 1], F32)
    b_ee_d = din("b_ee", [128, 2, 1], F32)
    b_d1p_d = din("b_d1p", [128, 2, 1], F32)
    i128_d = din("i128", [128, 128], BF16)
    iota_d = din("iota", [128, 128], F32)
    btot2_d = din("btot2", [1, HC], F32)
    abh_d = din("abh", [1, HEADS], F32)
    b_d2b_d = din("b_d2b", [128, 6], F32)
    sgn_b_d = din("sgn_b", [128, HC], BF16)
    sel4_d = din("sel4", [HEADS, HEADS * 128], F32)
    eaT_d = din("eaT", [TILES_PER_CORE, 7, EDGE_CAP], BF16)
    idxl_d = din("idxl", [TILES_PER_CORE, 128, IDX_COLS], I16)
    idxr_d = din("idxr", [TILES_PER_CORE, 128, IDX_COLS], I16)
    dstloc_d = din("dstloc", [TILES_PER_CORE, 128, CHUNKS], F32)
    out_d = nc.dram_tensor("out", [TILES_PER_CORE, 128, 6], F32,
                           kind="ExternalOutput")

    xl_shs = [nc.dram_tensor(f"xl_sh{s}", [NSHARD // 4, TBL_W], BF16)
              for s in range(4)]
    xl_tab = nc.dram_tensor("xl_tab", [8 * NSHARD, TBL_W], BF16,
                            addr_space="Shared")
    xr_tab = nc.dram_tensor("xr_tab", [MAX_LOC, TBL_W], BF16)

    with tile.TileContext(nc) as tc:
        # ---------------- prologue: build xl / xr tables ----------------
        with (
            tc.tile_pool(name="pro1", bufs=1) as p1,
            tc.tile_pool(name="pro", bufs=2) as ppool,
            tc.tile_pool(name="pro_ps", bufs=2, space="PSUM") as pps,
            tc.tile_pool(name="pro_ps2", bufs=2, space="PSUM") as pps2,
        ):
            w_ne_s = p1.tile([7, HID], BF16)
            w_l_s = p1.tile([128, 2, HC], BF16, tag="wl")
            w_r_s = p1.tile([128, 2, HC], BF16, tag="wr")
            w_al_s = p1.tile([128, 2, HEADS], BF16, tag="wal")
            w_ar_s = p1.tile([128, 2, HEADS], BF16, tag="war")
            b_ne_s = p1.tile([128, 2, 1], F32)
            xlocT_s = p1.tile([7, MAX_LOC], BF16, tag="xlocT")
            xshT_s = p1.tile([7, NSHARD], BF16, tag="xshT")
            hT = p1.tile([128, 2, NSHARD], BF16, tag="hT")
            hTl = p1.tile([128, 2, MAX_LOC], BF16, tag="hTl")
            ones1_p = p1.tile([1, 128], F32)
            btot2_p = p1.tile([1, HC], BF16)
            abh_p = p1.tile([1, HEADS], BF16)
            for dst_t, src_t in [
                (w_ne_s, w_ne_d), (w_l_s, w_l_d), (w_r_s, w_r_d),
                (w_al_s, w_al_d), (w_ar_s, w_ar_d), (b_ne_s, b_ne_d),
                (xlocT_s, xlocT_d), (xshT_s, xshT_d),
            ]:
                nc.sync.dma_start(dst_t[:], src_t[:])
            nc.vector.memset(ones1_p[:], 1.0)
            if has_btot:
                btf = ppool.tile([1, HC], F32, tag="btf")
                nc.sync.dma_start(btf[:], btot2_d[:])
                nc.vector.tensor_copy(btot2_p[:], btf[:])
                abf = ppool.tile([1, HEADS], F32, tag="abf")
                nc.sync.dma_start(abf[:], abh_d[:])
                nc.vector.tensor_copy(abh_p[:], abf[:])

            def enc_h(dst_tile, src_dram, src_tile, ncols):
                """dst[:, half, :] = relu(w_ne[:,half].T @ xT + b_ne)"""
                i = 0
                for c0 in range(0, ncols, 512):
                    n = min(512, ncols - c0)
                    if src_dram is not None:
                        st = ppool.tile([7, 512], BF16, tag="xTc")
                        nc.sync.dma_start(st[:, :n], src_dram[:, c0:c0 + n])
                    else:
                        st = None
                    for half in range(2):
                        ps = pps.tile([128, 512], F32, tag="ph")
                        nc.tensor.matmul(
                            ps[:, :n],
                            w_ne_s[:, half * 128:(half + 1) * 128],
                            st[:, :n] if st is not None
                            else src_tile[:, c0:c0 + n],
                            start=True, stop=True)
                        dst_ap = dst_tile[:, half, c0:c0 + n]
                        if i % 2 == 0:
                            nc.scalar.activation(
                                dst_ap, ps[:, :n], AF.Relu,
                                bias=b_ne_s[:, half, 0:1])
                        else:
                            nc.vector.tensor_scalar(
                                dst_ap, ps[:, :n],
                                b_ne_s[:, half, 0:1], 0.0,
                                op0=ALU.add, op1=ALU.max)
                        i += 1



            def build_table(tab_dram, h_tile, nrows, wmat, wvec,
                            post_chunk=None, row0=0):
                nt = (nrows + 127) // 128
                for it in range(nt):
                    r0 = it * 128
                    nr = min(128, nrows - r0)
                    ps = pps2.tile([128, 1152], F32, tag="pxl")
                    for half in range(2):
                        st_f = half == 0
                        sp_f = half == 1
                        for k in range(2):
                            nc.tensor.matmul(
                                ps[:nr, k * 512:(k + 1) * 512],
                                h_tile[:, half, row0 + r0:row0 + r0 + nr],
                                wmat[:, half, k * 512:(k + 1) * 512],
                                start=st_f, stop=sp_f and not has_btot)
                        nc.tensor.matmul(
                            ps[:nr, HC:HC + HEADS],
                            h_tile[:, half, row0 + r0:row0 + r0 + nr],
                            wvec[:, half, :], start=st_f,
                            stop=sp_f and not has_btot)
                    if has_btot:
                        nc.tensor.matmul(
                            ps[:nr, 0:512], ones1_p[0:1, :nr],
                            btot2_p[:, 0:512], start=False, stop=True)
                        nc.tensor.matmul(
                            ps[:nr, 512:1024], ones1_p[0:1, :nr],
                            btot2_p[:, 512:1024], start=False, stop=True)
                        nc.tensor.matmul(
                            ps[:nr, HC:HC + HEADS], ones1_p[0:1, :nr],
                            abh_p[:, :], start=False, stop=True)
                    stg = ppool.tile([128, TBL_USED], BF16, tag="stg")
                    if it % 2 == 0:
                        nc.scalar.activation(stg[:nr, :], ps[:nr, :TBL_USED],
                                             AF.Copy)
                    else:
                        nc.vector.tensor_copy(stg[:nr, :], ps[:nr, :TBL_USED])
                    for q in range(4):
                        qr0 = q * 32
                        qr1 = min(nr, qr0 + 32)
                        if qr1 > qr0:
                            nc.sync.dma_start(
                                tab_dram[r0 + qr0:r0 + qr1, :TBL_USED],
                                stg[qr0:qr1, :])
                    if post_chunk is not None:
                        post_chunk(it)

            SLAB_ROWS = NSHARD // 4
            SLAB_CHUNKS = SLAB_ROWS // 128

            enc_h(hT, None, xshT_s, NSHARD)
            for s in range(4):
                def kick(it, s=s):
                    if it + 1 != SLAB_CHUNKS:
                        return
                    nc.gpsimd.collective_compute(
                        "AllGather", ALU.bypass,
                        replica_groups=[list(range(N_CORES))],
                        ins=[xl_shs[s][:]],
                        outs=[xl_tab[s * N_CORES * SLAB_ROWS:
                                     (s + 1) * N_CORES * SLAB_ROWS, :]])
                build_table(xl_shs[s], hT, SLAB_ROWS, w_l_s, w_al_s,
                            post_chunk=kick, row0=s * SLAB_ROWS)
            enc_h(hTl, None, xlocT_s, MAX_LOC)
            build_table(xr_tab, hTl, MAX_LOC, w_r_s, w_ar_s)

        # ---------------- main loop over tiles ----------------
        with (
            tc.tile_pool(name="const", bufs=1) as cpool,
            tc.tile_pool(name="mn", bufs=2) as mpool,
            tc.tile_pool(name="gth", bufs=2) as gpool,
            tc.tile_pool(name="ps_s", bufs=1, space="PSUM") as ps_s_p,
            tc.tile_pool(name="ps_att", bufs=1, space="PSUM") as ps_att_p,
            tc.tile_pool(name="ps_agg", bufs=1, space="PSUM") as ps_agg_p,
            tc.tile_pool(name="ps_agB", bufs=1, space="PSUM") as ps_agB_p,
            tc.tile_pool(name="ps_agT", bufs=1, space="PSUM") as ps_agT_p,
            tc.tile_pool(name="ps_dt", bufs=1, space="PSUM") as ps_dt_p,
            tc.tile_pool(name="ps_post", bufs=1, space="PSUM") as ps_post_p,
        ):
            w_e_s = cpool.tile([128, 2, HC], BF16)
            w_ae_s = cpool.tile([128, 2, HEADS], BF16)
            w_d1_s = cpool.tile([128, 8, HID], BF16)
            w_d2_s = cpool.tile([128, 2, 6], BF16)
            w_ee_s = cpool.tile([7, HID], BF16)
            b_ee_s = cpool.tile([128, 2, 1], F32)
            b_d1p_s = cpool.tile([128, 2, 1], F32)
            i128_s = cpool.tile([128, 128], BF16)
            iota_s = cpool.tile([128, 128], F32)
            b_d2b_s = cpool.tile([128, 6], F32)
            sgn_b_s = cpool.tile([128, HC], BF16)
            for dst_t, src_t in [
                (w_e_s, w_e_d), (w_ae_s, w_ae_d), (w_d1_s, w_d1_d),
                (w_d2_s, w_d2_d), (w_ee_s, w_ee_d), (b_ee_s, b_ee_d),
                (b_d1p_s, b_d1p_d), (i128_s, i128_d),
                (iota_s, iota_d), (b_d2b_s, b_d2b_d), (sgn_b_s, sgn_b_d),
            ]:
                nc.sync.dma_start(dst_t[:], src_t[:])

            def stage_a(t):
                if phase < 1:
                    o_sb = mpool.tile([128, 6], F32, tag="o")
                    nc.vector.memset(o_sb[:], 0.5)
                    nc.sync.dma_start(out_d[t], o_sb[:])
                    return None
                ea_t = mpool.tile([7, EDGE_CAP], BF16, tag="ea")
                nc.sync.dma_start(ea_t[:], eaT_d[t])
                il_t = mpool.tile([128, IDX_COLS], I16, tag="il")
                nc.sync.dma_start(il_t[:], idxl_d[t])
                dl_t = mpool.tile([128, CHUNKS], F32, tag="dl")
                nc.sync.dma_start(dl_t[:], dstloc_d[t])

                xlg = gpool.tile([128, CHUNKS, TBL_W], BF16, tag="xlg")
                nc.gpsimd.dma_gather(
                    xlg[:], xl_tab[:], il_t[:], EDGE_CAP, EDGE_CAP, TBL_W)
                xrt = gpool.tile([128, TBL_USED], BF16, tag="xrt")
                nc.sync.dma_start(
                    xrt[:], xr_tab[t * 128:(t + 1) * 128, :TBL_USED])

                # encoder: eT[c, e] = relu(W_ee^T ea + b_ee)
                eT = mpool.tile([128, 2, EDGE_CAP], BF16, tag="eT")
                for half in range(2):
                    for c0 in range(0, EDGE_CAP, 512):
                        n = min(512, EDGE_CAP - c0)
                        pse = ps_post_p.tile([128, 512], F32, tag="post")
                        nc.tensor.matmul(
                            pse[:, :n],
                            w_ee_s[:, half * 128:(half + 1) * 128],
                            ea_t[:, c0:c0 + n], start=True, stop=True)
                        nc.scalar.activation(
                            eT[:, half, c0:c0 + n], pse[:, :n], AF.Relu,
                            bias=b_ee_s[:, half, 0:1])

                eq_tiles = []
                for j in range(CHUNKS):
                    eq = mpool.tile([128, 128], BF16, tag=f"eq{j}")
                    nc.vector.tensor_scalar(
                        eq[:], iota_s[:], dl_t[:, j:j + 1], None,
                        op0=ALU.is_equal)
                    eq_tiles.append(eq)
                eqT_tiles = []
                for j in range(CHUNKS):
                    ps_eqt = ps_att_p.tile([128, 128], BF16, tag="att")
                    nc.tensor.transpose(ps_eqt[:], eq_tiles[j][:], i128_s[:])
                    eqT = mpool.tile([128, 128], BF16, tag=f"eqT{j}")
                    nc.scalar.activation(eqT[:], ps_eqt[:], AF.Copy)
                    eqT_tiles.append(eqT)

                ps_dt = ps_dt_p.tile([128, HEADS], F32, tag="dt")
                ps_agA = ps_agg_p.tile([128, 512], F32, tag="aggA")
                ps_agB = ps_agB_p.tile([128, 512], F32, tag="aggB")
                ag_first = {}
                for j in range(CHUNKS):
                    ps_s = ps_s_p.tile([128, HC], F32, tag="s")
                    ps_att = ps_att_p.tile([128, HEADS], F32, tag="att")
                    for k in range(2):
                        for half in range(2):
                            nc.tensor.matmul(
                                ps_s[:, k * 512:(k + 1) * 512],
                                eT[:, half, j * 128:(j + 1) * 128],
                                w_e_s[:, half, k * 512:(k + 1) * 512],
                                start=(half == 0), stop=False)
                        nc.tensor.matmul(
                            ps_s[:, k * 512:(k + 1) * 512], i128_s[:],
                            xlg[:, j, k * 512:(k + 1) * 512],
                            start=False, stop=False)
                        nc.tensor.matmul(
                            ps_s[:, k * 512:(k + 1) * 512], eqT_tiles[j][:],
                            xrt[:, k * 512:(k + 1) * 512],
                            start=False, stop=True)
                    for half in range(2):
                        nc.tensor.matmul(
                            ps_att[:],
                            eT[:, half, j * 128:(j + 1) * 128],
                            w_ae_s[:, half, :],
                            start=(half == 0), stop=False,
                            skip_group_check=True)
                    # ar[dst] via EqT expansion folded into the ps_att group
                    nc.tensor.matmul(
                        ps_att[:], eqT_tiles[j][:], xrt[:, HC:HC + HEADS],
                        start=False, stop=True, skip_group_check=True)
                    # lin = 0.2*(al + ar + att.e_h)  (0.2 folded host-side)
                    lin = mpool.tile([128, HEADS], F32, tag="lin")
                    nc.vector.tensor_tensor(
                        lin[:], xlg[:, j, HC:HC + HEADS], ps_att[:], ALU.add)
                    # relu part of leaky (|att|,0.8 folded into tables):
                    # logits = lin + sum_c sign_c * relu(s'_c)
                    zr = mpool.tile([128, HC], BF16, tag="zr")
                    for k in range(2):
                        nc.scalar.activation(
                            zr[:, k * 512:(k + 1) * 512],
                            ps_s[:, k * 512:(k + 1) * 512], AF.Relu)
                    lgj = mpool.tile([128, HEADS], F32, tag="lgj")
                    z2 = mpool.tile([128, HC], BF16, tag="z2")
                    for h in range(HEADS):
                        nc.vector._custom_dve(
                            CTTR,
                            out=z2[:, h * OUTC:(h + 1) * OUTC],
                            in0=zr[:, h * OUTC:(h + 1) * OUTC],
                            in1=sgn_b_s[:, h * OUTC:(h + 1) * OUTC],
                            s0=lin[:, h:h + 1], s1=1.0,
                            accum_out=lgj[:, h:h + 1])
                    expv = mpool.tile([128, HEADS], F32, tag="ex")
                    nc.scalar.activation(expv[:], lgj[:], AF.Exp)
                    expb = mpool.tile([128, HEADS], BF16, tag="exb")
                    nc.vector.tensor_copy(expb[:], expv[:])
                    nc.tensor.matmul(
                        ps_dt[:], eq_tiles[j][:], expb[:],
                        start=(j == 0), stop=(j == CHUNKS - 1))
                    for h in range(HEADS):
                        mjh = mpool.tile([128, 128], BF16, tag="mjh")
                        nc.vector.tensor_scalar(
                            mjh[:], eq_tiles[j][:], expv[:, h:h + 1],
                            None, op0=ALU.mult)
                        ps_ag = ps_agA if h < 2 else ps_agB
                        hh = h % 2
                        mm = nc.tensor.matmul(
                            ps_ag[:, hh * OUTC:(hh + 1) * OUTC], mjh[:],
                            xlg[:, j, h * OUTC:(h + 1) * OUTC],
                            start=(j == 0 and hh == 0),
                            stop=(j == CHUNKS - 1 and hh == 1),
                            skip_group_check=True)
                        # zero-region ordering within each shared psum bank
                        if j == 0 and hh == 0:
                            ag_first[h // 2] = mm
                        elif j == 0 and hh == 1:
                            tile.add_dep_helper(
                                mm.ins, ag_first[h // 2].ins,
                                reason="psum zero-region order")
                        elif j == CHUNKS - 1 and hh == 0:
                            ag_first[10 + h // 2] = mm
                        elif j == CHUNKS - 1 and hh == 1:
                            tile.add_dep_helper(
                                mm.ins, ag_first[10 + h // 2].ins,
                                reason="psum stop order")

                den = mpool.tile([128, HEADS], F32, tag="den")
                nc.vector.tensor_scalar(
                    den[:], ps_dt[:], 1e-30, None, op0=ALU.max)
                rec = mpool.tile([128, HEADS], F32, tag="rec")
                nc.vector.reciprocal(rec[:], den[:])

                aggs = mpool.tile([128, HC], BF16, tag="aggs")
                for h in range(HEADS):
                    ps_ag = ps_agA if h < 2 else ps_agB
                    hh = h % 2
                    nc.scalar.activation(
                        aggs[:, h * OUTC:(h + 1) * OUTC],
                        ps_ag[:, hh * OUTC:(hh + 1) * OUTC],
                        AF.Copy, scale=rec[:, h:h + 1])
                return aggs

            def stage_b(aggs, t):
                # transpose agg -> c-major, then decoder MLP + sigmoid
                ps_at = ps_agT_p.tile([128, HC], BF16, tag="agT")
                for g in range(8):
                    nc.tensor.transpose(
                        ps_at[:, g * 128:(g + 1) * 128],
                        aggs[:, g * 128:(g + 1) * 128], i128_s[:])
                aggT = mpool.tile([128, 8, 128], BF16, tag="aggT")
                nc.vector.tensor_copy(
                    aggT[:].rearrange("p a b -> p (a b)"), ps_at[:])

                ps_d1 = ps_post_p.tile([128, HID], F32, tag="post")
                for co in range(2):
                    for ci in range(8):
                        nc.tensor.matmul(
                            ps_d1[:, co * 128:(co + 1) * 128],
                            w_d1_s[:, ci, co * 128:(co + 1) * 128],
                            aggT[:, ci, :],
                            start=(ci == 0), stop=(ci == 7))
                d1 = mpool.tile([128, 2, 128], BF16, tag="d1")
                for co in range(2):
                    nc.scalar.activation(
                        d1[:, co, :], ps_d1[:, co * 128:(co + 1) * 128],
                        AF.Relu, bias=b_d1p_s[:, co, 0:1])

                ps_o = ps_att_p.tile([128, HEADS + 2], F32, tag="att")
                for co in range(2):
                    nc.tensor.matmul(
                        ps_o[:, 0:6], d1[:, co, :], w_d2_s[:, co, :],
                        start=(co == 0), stop=(co == 1))
                o_sb = mpool.tile([128, 6], F32, tag="o")
                if has_bd2:
                    tmp_o = mpool.tile([128, 6], F32, tag="o2")
                    nc.vector.tensor_tensor(
                        tmp_o[:], ps_o[:, 0:6], b_d2b_s[:], ALU.add)
                    nc.scalar.activation(o_sb[:], tmp_o[:], AF.Sigmoid)
                else:
                    nc.scalar.activation(o_sb[:], ps_o[:, 0:6], AF.Sigmoid)
                nc.sync.dma_start(out_d[t], o_sb[:])

            pending = None
            for t in range(TILES_PER_CORE):
                ag = stage_a(t)
                if pending is not None:
                    stage_b(*pending)
                if ag is not None:
                    pending = (ag, t)
                else:
                    pending = None
            if pending is not None:
                stage_b(*pending)

    nc.compile()
    return nc


# --------------------------------------------------------------------------
# entry point
# --------------------------------------------------------------------------

def kernel(**inputs):
    in_maps, meta, flags = _host_prep(inputs)
    if flags not in _CACHE:
        _CACHE[flags] = _build(flags)
    nc = _CACHE[flags]

    from concourse.bass_utils import run_bass_kernel_spmd
    res = run_bass_kernel_spmd(
        nc, in_maps, core_ids=list(range(N_CORES)),
        trace=os.environ.get("BASS_KERNEL_TRACE", "0") == "1")
    kernel.last_exec_time_ns = res.exec_time_ns

    out = np.zeros((N_NODES, 6), np.float32)
    for c in range(N_CORES):
        stage = res.results[c]["out"]  # [T, 128, 6]
        for t, (n0, n1) in enumerate(meta[c]):
            if n1 > n0:
                out[n0:n1] = stage[t, :n1 - n0, :]
    return out


kernel.last_exec_time_ns = None

